# revision 1
# baseline (speedup 1.0000x reference)
"""MoE (DbrxExperts) expert-parallel Trainium2 kernel.

Strategy:
  - Host: compute per-(expert,token) combine weights cw from top_experts /
    top_weights, gather each expert's routed tokens, pad to a common
    capacity C, and pre-transpose operands so the device kernel needs no
    on-chip transposes.
  - Device (8 cores, SPMD, 2 experts/core): per expert
        gate_T = W1T_blocks^T @ XT     [F, C]   (contract H)
        up_T   = V1T_blocks^T @ XT     [F, C]
        hact_T = silu(gate_T) * up_T   [F, C]   (ACT + DVE)
        down   = hact_T_blocks^T @ W2  [C, H]   (contract F)
    All fp32. Output in natural [C, H] layout.
  - Host: out[tokens_e] += down_e * cw_e  (scaling folded into combine).
"""

import numpy as np
from contextlib import ExitStack

N_CORES = 8
B, S, H = 4, 2048, 1024
F, E = 2048, 16
T = B * S
E_LOC = E // N_CORES  # experts per core

P = 128
HT = H // P   # 8  h-tiles
FT = F // P   # 16 f-tiles

TRACE = False          # test.py sets this for profiled runs
LAST_RESULT = None     # BassKernelResults of last run (for test.py)

_nc_cache = {}


def _chunks(C):
    out = []
    c0 = 0
    while c0 < C:
        s = min(1024, C - c0)
        out.append((c0, s))
        c0 += s
    return out


def _build_nc(C):
    import concourse.tile as tile
    from concourse import bacc, mybir

    nc = bacc.Bacc("TRN2", target_bir_lowering=False, debug=False,
                   enable_asserts=False, num_devices=N_CORES)
    dt = mybir.dt.float32
    SILU = mybir.ActivationFunctionType.Silu

    xt = nc.dram_tensor("xt", [E_LOC, H, C], dt, kind="ExternalInput").ap()
    w1t = nc.dram_tensor("w1t", [E_LOC, H, F], dt, kind="ExternalInput").ap()
    v1t = nc.dram_tensor("v1t", [E_LOC, H, F], dt, kind="ExternalInput").ap()
    w2 = nc.dram_tensor("w2", [E_LOC, F, H], dt, kind="ExternalInput").ap()
    y = nc.dram_tensor("y", [E_LOC, C, H], dt, kind="ExternalOutput").ap()

    with tile.TileContext(nc) as tc:
        with ExitStack() as ctx:
            xt_pool = ctx.enter_context(tc.tile_pool(name="xt", bufs=HT))
            wst_pool = ctx.enter_context(tc.tile_pool(name="wst", bufs=3))
            w2_pool = ctx.enter_context(tc.tile_pool(name="w2sb", bufs=FT))
            hact_pool = ctx.enter_context(tc.tile_pool(name="hact", bufs=FT))
            silu_pool = ctx.enter_context(tc.tile_pool(name="silu", bufs=2))
            out_pool = ctx.enter_context(tc.tile_pool(name="out", bufs=2))
            g_pool = ctx.enter_context(tc.tile_pool(name="g_ps", bufs=2, space="PSUM"))
            u_pool = ctx.enter_context(tc.tile_pool(name="u_ps", bufs=2, space="PSUM"))
            d_pool = ctx.enter_context(tc.tile_pool(name="d_ps", bufs=2, space="PSUM"))

            for e in range(E_LOC):
                # W2 tiles resident for this expert (natural [f, h] layout)
                w2_sb = []
                for ft in range(FT):
                    t = w2_pool.tile([P, H], dt, tag="w2")
                    nc.sync.dma_start(t[:], w2[e, ft * P:(ft + 1) * P, :])
                    w2_sb.append(t)

                for (c0, S_) in _chunks(C):
                    # XT chunk: 8 tiles [128, S_], partition = h within tile
                    xt_sb = []
                    for ht in range(HT):
                        t = xt_pool.tile([P, S_], dt, tag="xt")
                        nc.sync.dma_start(
                            t[:], xt[e, ht * P:(ht + 1) * P, c0:c0 + S_])
                        xt_sb.append(t)

                    # GEMM1/2 + GLU -> hact_T tiles [128, S_] per f-tile
                    hact_sb = []
                    for ft in range(FT):
                        h_t = hact_pool.tile([P, S_], dt, tag="hact")
                        w1s = wst_pool.tile([P, HT, P], dt, tag="wst")
                        nc.sync.dma_start(
                            w1s[:],
                            w1t[e, :, ft * P:(ft + 1) * P]
                            .rearrange("(o p) f -> p o f", p=P))
                        v1s = wst_pool.tile([P, HT, P], dt, tag="wst")
                        nc.sync.dma_start(
                            v1s[:],
                            v1t[e, :, ft * P:(ft + 1) * P]
                            .rearrange("(o p) f -> p o f", p=P))
                        for s5 in range(0, S_, 512):
                            w_ = min(512, S_ - s5)
                            g_ps = g_pool.tile([P, w_], dt, tag="g")
                            u_ps = u_pool.tile([P, w_], dt, tag="u")
                            for ht in range(HT):
                                nc.tensor.matmul(
                                    g_ps[:], w1s[:, ht, :],
                                    xt_sb[ht][:, s5:s5 + w_],
                                    start=(ht == 0), stop=(ht == HT - 1))
                            for ht in range(HT):
                                nc.tensor.matmul(
                                    u_ps[:], v1s[:, ht, :],
                                    xt_sb[ht][:, s5:s5 + w_],
                                    start=(ht == 0), stop=(ht == HT - 1))
                            sl = silu_pool.tile([P, w_], dt, tag="sl")
                            nc.scalar.activation(sl[:], g_ps[:], SILU)
                            nc.vector.tensor_mul(
                                h_t[:, s5:s5 + w_], sl[:], u_ps[:])
                        hact_sb.append(h_t)

                    # GEMM3: down[c, h] accumulated over f-tiles
                    for ct in range(S_ // P):
                        d_ps = d_pool.tile([P, H], dt, tag="d")
                        for ft in range(FT):
                            for hh in range(0, H, 512):
                                nc.tensor.matmul(
                                    d_ps[:, hh:hh + 512],
                                    hact_sb[ft][:, ct * P:(ct + 1) * P],
                                    w2_sb[ft][:, hh:hh + 512],
                                    start=(ft == 0), stop=(ft == FT - 1))
                        o_t = out_pool.tile([P, H], dt, tag="o")
                        nc.any.tensor_copy(o_t[:], d_ps[:])
                        nc.sync.dma_start(
                            y[e, c0 + ct * P:c0 + (ct + 1) * P, :], o_t[:])
    nc.compile()
    return nc


def _get_nc(C):
    if C not in _nc_cache:
        _nc_cache[C] = _build_nc(C)
    return _nc_cache[C]


def kernel(x, weights, top_weights, top_experts, w1, v1, w2):
    global LAST_RESULT
    x = np.asarray(x, dtype=np.float32)
    top_weights = np.asarray(top_weights, dtype=np.float32)
    top_experts = np.asarray(top_experts).astype(np.int64)
    w1 = np.asarray(w1, dtype=np.float32)
    v1 = np.asarray(v1, dtype=np.float32)
    w2 = np.asarray(w2, dtype=np.float32)

    xf = x.reshape(T, H)

    # combine weights per (token, expert); duplicate slots sum
    cw = np.zeros((T, E), dtype=np.float32)
    np.add.at(cw, (np.arange(T)[:, None], top_experts), top_weights)

    idx = [np.nonzero(cw[:, e])[0] for e in range(E)]
    counts = [len(i) for i in idx]
    C = max(128, -(-max(counts) // P) * P)

    nc = _get_nc(C)

    # per-core input maps
    in_maps = []
    for m in range(N_CORES):
        XT = np.zeros((E_LOC, H, C), dtype=np.float32)
        for le in range(E_LOC):
            e = m * E_LOC + le
            XT[le, :, :counts[e]] = xf[idx[e]].T
        in_maps.append({
            "xt": XT,
            "w1t": np.ascontiguousarray(
                w1[m * E_LOC:(m + 1) * E_LOC].transpose(0, 2, 1)),
            "v1t": np.ascontiguousarray(
                v1[m * E_LOC:(m + 1) * E_LOC].transpose(0, 2, 1)),
            "w2": np.ascontiguousarray(w2[m * E_LOC:(m + 1) * E_LOC]),
        })

    from concourse.bass_utils import run_bass_kernel_spmd
    res = run_bass_kernel_spmd(nc, in_maps, list(range(N_CORES)), trace=TRACE)
    LAST_RESULT = res

    out = np.zeros((T, H), dtype=np.float32)
    for m in range(N_CORES):
        ym = res.results[m]["y"]
        for le in range(E_LOC):
            e = m * E_LOC + le
            n = counts[e]
            if n:
                out[idx[e]] += ym[le, :n, :] * cw[idx[e], e][:, None]
    return out.reshape(B, S, H)
